# revision 38
# baseline (speedup 1.0000x reference)
"""Causal multi-head self-attention with RoPE on 8 Trainium2 NeuronCores.

Problem: x[2,2048,2048] fp32, wq/wk/wv[2048,2048] fp32 (Linear [out,in]),
H=16 heads, dk=128, causal softmax attention, RoPE(theta=1e4).

Sharding (hybrid tensor/data parallel, no collectives): core c handles
batch b=c//4 and head group hg=c%4 (4 heads = 512 output features).
The host concatenates the 8 per-core outputs.

Device kernel (per core, all matmuls bf16 with fp32 PSUM accumulation):
  - All inputs are host-packed into [128, N] SBUF-image layouts so every
    DMA moves fat (>=4KB) contiguous per-partition rows: x per t-block
    [p, kt*512], wq/wk head-major [p, h*kt*128], wv kt-major [p, kt*512].
  - DMA plan: each transfer carries ~5.5us of dead setup serialized on
    its queue, so ~1-2MB transfers are emitted in exactly the order the
    PE consumes them, split across the two HWDGE queues (sync, scalar);
    the slow SWDGE queue (gpsimd) carries the tables, half of wv, and
    the last x block. Output tiles go back on sync (idle by then).
  - t-block-major compute order so 2MB of x unlocks all-head work, with
    causal attention interleaved: q-block tb only needs k/v blocks <=
    tb, so attention(h, qb=tb) runs inside tb's phase, and from tb1 on
    each attention block is paired with a K-projection chain to balance
    ScalarE exp (the attention bottleneck) against PE work.
  - QKV projections: qT/kT in [dk, t] layout, v in [t, dk] with a ones
    column (129 wide) so the PV matmul also yields the softmax denom.
  - RoPE via rotate-half entirely on the vector engine: a
    stream_shuffle pair-swap (the 32-lane crossbar applies the i^1 mask
    to each 32-partition group) plus 2x-mode bf16 tensor_mul/add; the
    rotation sign and 1/sqrt(dk) are folded into the s2 table / wq on
    host.
  - Attention in s^T = [keys, queries] layout; off-diagonal key tiles
    processed in PAIRS whose scores land in one 2-bank PSUM tile so a
    single 1024-wide exp covers both (halves ScalarE's per-ACTIVATE
    overhead); diagonal tiles multiply a [128,128] triangle mask into
    p^T and skip fully-masked query subtiles.
  - PE warm-up: dummy 512-wide matmuls spin the tensor engine through
    the HAM cold window while the first input DMAs land.
"""
import os
import sys
import time

# a wedged device from a prior process recovers on reset; must be set
# before the first jax/neuron import in this process
os.environ.setdefault("NEURON_RT_RESET_CORES", "1")

sys.path.insert(0, "/opt/trn_rl_repo")

import numpy as np
import ml_dtypes

import concourse.bass as bass
import concourse.bacc as bacc
import concourse.mybir as mybir
import concourse.tile as tile
from concourse.bass_utils import run_bass_kernel_spmd

B, S, D = 2, 2048, 2048
H, DK = 16, 128
N_CORES = 8
HPC = 4            # heads per core
FPC = HPC * DK     # features per core (512)
P = 128            # partitions
KT = D // P        # contraction k-tiles (16)
TBW = 512          # token-block width for projections
NTB = S // TBW     # 4 t-blocks
NQT = S // P       # 16 query tiles of 128
THETA = 10000.0
WARM_MMS = 44      # dummy 512-wide matmuls covering the initial DMA wait

bf16 = ml_dtypes.bfloat16
_mult = mybir.AluOpType.mult
_add = mybir.AluOpType.add

_PROGRAM_CACHE = {}


def _build_program():
    dt = mybir.dt
    nc = bacc.Bacc("TRN2", target_bir_lowering=False, debug=False,
                   num_devices=N_CORES)

    x_d = [nc.dram_tensor(f"x{i}", [P, KT * TBW], dt.bfloat16,
                          kind="ExternalInput").ap() for i in range(NTB)]
    wq_d = nc.dram_tensor("wq", [P, HPC * KT * DK], dt.bfloat16,
                          kind="ExternalInput").ap()
    wk_d = nc.dram_tensor("wk", [P, HPC * KT * DK], dt.bfloat16,
                          kind="ExternalInput").ap()
    wv_d = nc.dram_tensor("wv", [P, KT * FPC], dt.bfloat16,
                          kind="ExternalInput").ap()
    c2_d = nc.dram_tensor("c2", [P, S], dt.bfloat16, kind="ExternalInput").ap()
    s2_d = nc.dram_tensor("s2", [P, S], dt.bfloat16, kind="ExternalInput").ap()
    tri_d = nc.dram_tensor("tri", [P, P], dt.bfloat16, kind="ExternalInput").ap()
    out_d = nc.dram_tensor("out", [S, FPC], dt.float32, kind="ExternalOutput").ap()

    ts = bass.ts

    with tile.TileContext(nc) as tc:
        with (
            tc.tile_pool(name="const", bufs=1) as cpool,
            tc.tile_pool(name="work", bufs=4) as wpool,
            tc.tile_pool(name="small", bufs=6) as smpool,
            tc.tile_pool(name="ppsum", bufs=2,
                         space=bass.MemorySpace.PSUM) as ppsum,
            tc.tile_pool(name="spsum", bufs=2,
                         space=bass.MemorySpace.PSUM) as spsum,
            tc.tile_pool(name="opsum", bufs=1,
                         space=bass.MemorySpace.PSUM) as opsum,
            tc.tile_pool(name="ppool", bufs=4) as ppool,
        ):
            # --- persistent SBUF tensors (flat, with compute views) ---
            xts = [cpool.tile([P, KT * TBW], dt.bfloat16,
                              tag=f"xt{i}", name=f"xt{i}") for i in range(NTB)]
            xr = [t.rearrange("p (k t) -> p k t", t=TBW) for t in xts]
            wq_sb = cpool.tile([P, HPC * KT * DK], dt.bfloat16, tag="wq")
            wk_sb = cpool.tile([P, HPC * KT * DK], dt.bfloat16, tag="wk")
            wv_sb = cpool.tile([P, KT * FPC], dt.bfloat16, tag="wv")
            wq_r = wq_sb.rearrange("p (h k o) -> p h k o", k=KT, o=DK)
            wk_r = wk_sb.rearrange("p (h k o) -> p h k o", k=KT, o=DK)
            wv_r = wv_sb.rearrange("p (k o) -> p k o", o=FPC)
            c2_sb = cpool.tile([P, S], dt.bfloat16, tag="c2")
            s2_sb = cpool.tile([P, S], dt.bfloat16, tag="s2")
            tri_sb = cpool.tile([P, P], dt.bfloat16, tag="tri")
            qT_sb = cpool.tile([P, HPC * S], dt.bfloat16, tag="qT")
            kT_sb = cpool.tile([P, HPC * S], dt.bfloat16, tag="kT")
            # v with ones column: index n = h*NQT + tt -> [128 tokens, 129]
            v_sb = cpool.tile([P, HPC * NQT * (DK + 1)], dt.bfloat16, tag="v")
            v_rr = v_sb.rearrange("p (n c) -> p n c", c=DK + 1)
            warm_sb = cpool.tile([P, TBW], dt.bfloat16, tag="warm")

            # --- DMA schedule ---
            # Each transfer costs ~5.5us of dead setup time serialized on
            # its queue (the first 2-3 pre-queued ones pipeline tighter),
            # so: ~1-2MB transfers in consumption order. Two HWDGE queues
            # (sync, scalar) carry the critical path; the slow SWDGE
            # queue (gpsimd, ~65 GB/s) carries the tables, half of wv,
            # and the last x block. Memsets go on the otherwise-idle
            # vector engine so the PE warmup starts right after the NEFF
            # preamble.
            HC = KT * TBW // 2        # half of an x block
            WQH = 2 * KT * DK         # two heads' slice of wq/wk
            nc.vector.memset(warm_sb[:], 0.0)
            nc.vector.memset(v_rr[:, :, DK:DK + 1], 1.0)
            WVH = KT * FPC // 2       # wv first half (kt 0-7)
            nc.gpsimd.dma_start(tri_sb[:], tri_d[:])
            nc.gpsimd.dma_start(c2_sb[:], c2_d[:])
            nc.gpsimd.dma_start(s2_sb[:], s2_d[:])
            nc.gpsimd.dma_start(wv_sb[:, 0:WVH], wv_d[:, 0:WVH])
            nc.gpsimd.dma_start(xts[3][:], x_d[3][:])
            nc.sync.dma_start(wq_sb[:, 0:WQH], wq_d[:, 0:WQH])
            nc.scalar.dma_start(xts[0][:, 0:HC], x_d[0][:, 0:HC])
            nc.sync.dma_start(xts[0][:, HC:], x_d[0][:, HC:])
            nc.scalar.dma_start(wk_sb[:, 0:WQH], wk_d[:, 0:WQH])
            nc.sync.dma_start(wq_sb[:, WQH:], wq_d[:, WQH:])
            nc.scalar.dma_start(wv_sb[:, WVH:], wv_d[:, WVH:])
            nc.sync.dma_start(wk_sb[:, WQH:], wk_d[:, WQH:])
            nc.scalar.dma_start(xts[1][:], x_d[1][:])
            nc.sync.dma_start(xts[2][:], x_d[2][:])
            # tb0 consumes heads 0/1 of wq+wk before heads 2/3, giving
            # the '23' weight transfers ~7us of extra arrival slack
            tb0_order = [(0, "q"), (1, "q"), (0, "k"), (1, "k"),
                         (2, "q"), (3, "q"), (2, "k"), (3, "k")]

            # preload the exp table set (~2.7us) while ScalarE is idle
            dumm = smpool.tile([P, 1], dt.float32, tag="rec", name="dumm")
            nc.scalar.activation(dumm[:], warm_sb[:, 0:1],
                                 mybir.ActivationFunctionType.Exp)

            # PE warm-up: dependency-free matmuls spin the tensor engine
            # out of its cold HAM state while the first input DMAs land
            warm_ps = ppsum.tile([P, TBW], dt.float32, tag="qps",
                                 name="warm_ps")
            for _ in range(WARM_MMS):
                nc.tensor.matmul(warm_ps[:], warm_sb[:, 0:P], warm_sb[:],
                                 start=True, stop=True)

            # rotate-half via DVE stream_shuffle: the 32-lane crossbar
            # applies the same permutation to each 32-partition group, so
            # the adjacent-pair swap (i^1) works across all 128
            # partitions; the rotation sign is folded into s2 host-side.
            # tensor_mul/add (not scalar_tensor_tensor): all-bf16-SBUF
            # tensor_tensor has a 2x DVE perf mode, STT is 1x-capped.
            swap_mask = [i ^ 1 for i in range(32)]

            def qk_one(tb, h, w_r, dest):
                qps = ppsum.tile([P, TBW], dt.float32, tag="qps", name="qps")
                for kt in range(KT):
                    nc.tensor.matmul(
                        qps[:], w_r[:, h, kt, :], xr[tb][:, kt, :],
                        start=(kt == 0), stop=(kt == KT - 1))
                qb_ = wpool.tile([P, TBW], dt.bfloat16, tag="qb", name="qb")
                nc.vector.tensor_copy(qb_[:], qps[:])
                qsw = wpool.tile([P, TBW], dt.bfloat16, tag="qsw",
                                 name="qsw")
                nc.vector.stream_shuffle(qsw[:], qb_[:], swap_mask)
                t1 = wpool.tile([P, TBW], dt.bfloat16, tag="t1", name="t1")
                nc.vector.tensor_mul(t1[:], qb_[:], c2_sb[:, ts(tb, TBW)])
                t2 = wpool.tile([P, TBW], dt.bfloat16, tag="t2", name="t2")
                nc.vector.tensor_mul(t2[:], qsw[:], s2_sb[:, ts(tb, TBW)])
                hq = h * S
                nc.vector.tensor_add(
                    dest[:, hq + tb * TBW:hq + (tb + 1) * TBW],
                    t1[:], t2[:])

            def v_one(tb):
                for tsub in range(4):
                    tt = tb * 4 + tsub
                    vps = ppsum.tile([P, FPC], dt.float32,
                                     tag="qps", name="vps")
                    for kt in range(KT):
                        nc.tensor.matmul(
                            vps[:],
                            xr[tb][:, kt, ts(tsub, P)],
                            wv_r[:, kt, :],
                            start=(kt == 0), stop=(kt == KT - 1))
                    for h in range(HPC):
                        nc.vector.tensor_copy(v_rr[:, h * NQT + tt, 0:DK],
                                              vps[:, ts(h, DK)])

            def attn_block(h, qb, last=False):
                # two accumulators share a PSUM bank pair: one start
                # pending-zeroes the region, one stop (on the higher
                # subtile, which always finishes later) releases it
                hq = h * S
                o01 = opsum.tile([P, 2, DK + 1], dt.float32,
                                 tag="o01", name="o01")
                o23 = opsum.tile([P, 2, DK + 1], dt.float32,
                                 tag="o23", name="o23")
                oap = [o01[:, 0, :], o01[:, 1, :],
                       o23[:, 0, :], o23[:, 1, :]]
                # full (off-diagonal) key tiles in pairs: scores land in a
                # 2-bank PSUM tile so ONE 1024-wide exp covers both,
                # halving ScalarE's ~293ns fixed cost per ACTIVATE
                for pp in range(2 * qb):
                    sp2 = spsum.tile([P, 2, TBW], dt.float32,
                                     tag="sps", name="sp2")
                    p2 = ppool.tile([P, 2, TBW], dt.bfloat16,
                                    tag="pT", name="p2")
                    for j in range(2):
                        kt = 2 * pp + j
                        nc.tensor.matmul(
                            sp2[:, j, :],
                            kT_sb[:, hq + kt * P:hq + (kt + 1) * P],
                            qT_sb[:, hq + qb * TBW:hq + (qb + 1) * TBW],
                            start=True, stop=True)
                    nc.scalar.activation(
                        p2[:, :, :], sp2[:, :, :],
                        mybir.ActivationFunctionType.Exp)
                    for j in range(2):
                        kt = 2 * pp + j
                        for osub in range(4):
                            nc.tensor.matmul(
                                oap[osub],
                                p2[:, j, osub * P:(osub + 1) * P],
                                v_rr[:, h * NQT + kt, :],
                                start=(kt == 0 and osub % 2 == 0),
                                stop=False)
                # diagonal tiles: singles, triangle-masked, masked query
                # subtiles skipped
                for od in range(4):
                    kt = 4 * qb + od
                    lo = od * P
                    sp2 = spsum.tile([P, 2, TBW], dt.float32,
                                     tag="sps", name="spd")
                    p2 = ppool.tile([P, 2, TBW], dt.bfloat16,
                                    tag="pT", name="pd")
                    sps = sp2[:, 0, :]
                    pT = p2[:, 0, :]
                    nc.tensor.matmul(
                        sps[:, lo:TBW],
                        kT_sb[:, hq + kt * P:hq + (kt + 1) * P],
                        qT_sb[:, hq + qb * TBW + lo:hq + (qb + 1) * TBW],
                        start=True, stop=True)
                    nc.scalar.activation(
                        pT[:, lo:TBW], sps[:, lo:TBW],
                        mybir.ActivationFunctionType.Exp)
                    nc.vector.tensor_mul(
                        pT[:, od * P:(od + 1) * P],
                        pT[:, od * P:(od + 1) * P], tri_sb[:])
                    for osub in range(od, 4):
                        qt = 4 * qb + osub
                        nc.tensor.matmul(
                            oap[osub],
                            pT[:, osub * P:(osub + 1) * P],
                            v_rr[:, h * NQT + kt, :],
                            start=(kt == 0 and osub % 2 == 0),
                            stop=(kt == qt and osub % 2 == 1))
                # last block: osub3 emitted before osub2 so its
                # reciprocal clears the vector FIFO first and the scalar
                # tail chain (mul3+dma3) overlaps osub2's vector chain
                for osub in ((0, 1, 3, 2) if last else range(4)):
                    qt = 4 * qb + osub
                    rec = smpool.tile([P, 1], dt.float32,
                                      tag="rec", name="rec")
                    nc.vector.reciprocal(rec[:], oap[osub][:, DK:DK + 1])
                    osb = smpool.tile([P, DK], dt.float32,
                                      tag="osb", name="osb")
                    if last and osub == 3:
                        # kernel tail: run the final normalize + out-DMA
                        # on ScalarE (idle after its last exp) in
                        # parallel with osub 2's vector/sync chain
                        nc.scalar.mul(osb[:], oap[osub][:, 0:DK], rec[:])
                        nc.scalar.dma_start(out_d[ts(qt, P), ts(h, DK)],
                                            osb[:])
                    else:
                        nc.vector.tensor_scalar_mul(
                            osb[:], oap[osub][:, 0:DK], rec[:])
                        nc.sync.dma_start(out_d[ts(qt, P), ts(h, DK)],
                                          osb[:])

            # t-block-major; from tb1 on, K chains interleave between
            # attention blocks so ScalarE exp (the attention-phase
            # bottleneck) overlaps K-chain PE work. tb0 keeps K before V
            # because wv arrives after wk.
            for h, which in tb0_order:
                if which == "q":
                    qk_one(0, h, wq_r, qT_sb)
                else:
                    qk_one(0, h, wk_r, kT_sb)
            v_one(0)
            for h in range(HPC):
                attn_block(h, 0)
            for tb in range(1, NTB):
                for h in range(HPC):
                    qk_one(tb, h, wq_r, qT_sb)
                v_one(tb)
                for h in range(HPC):
                    qk_one(tb, h, wk_r, kT_sb)
                    attn_block(h, tb,
                               last=(tb == NTB - 1 and h == HPC - 1))

    nc.compile()
    return nc


def _host_tables():
    pos = np.arange(S, dtype=np.float64)
    i = np.arange(DK // 2, dtype=np.float64)
    inv_freq = THETA ** (-2.0 * i / DK)
    ang = pos[None, :] * inv_freq[:, None]          # [64, S]
    c2 = np.repeat(np.cos(ang), 2, axis=0).astype(bf16)   # [128, S]
    # rotate-half on device is a pure pair swap (stream_shuffle); the
    # -sin on even rows is folded into the table here
    s2 = np.repeat(np.sin(ang), 2, axis=0)
    s2[0::2] *= -1.0
    s2 = s2.astype(bf16)
    tri = (np.arange(P)[:, None] <= np.arange(P)[None, :]).astype(np.float32)
    return c2, s2, tri.astype(bf16)


def _pack_x(xb):
    # [p, kt*TBW + t] = xb[tb*TBW + t, kt*128 + p], one array per t-block
    arr = xb.reshape(NTB, TBW, KT, P).transpose(0, 3, 2, 1)
    return np.ascontiguousarray(arr).astype(bf16).reshape(NTB, P, KT * TBW)


def _pack_w_headmajor(w):
    # [p, h*KT*DK + kt*DK + o] = w.T[kt*128 + p, h*DK + o]
    arr = w.T.reshape(KT, P, HPC, DK).transpose(1, 2, 0, 3)
    return np.ascontiguousarray(arr).astype(bf16).reshape(P, HPC * KT * DK)


def _pack_w_ktmajor(w):
    # [p, kt*FPC + o] = w.T[kt*128 + p, o]
    arr = w.T.reshape(KT, P, FPC).transpose(1, 0, 2)
    return np.ascontiguousarray(arr).astype(bf16).reshape(P, KT * FPC)


def kernel(x, wq, wk, wv):
    x = np.asarray(x, dtype=np.float32)
    wq = np.asarray(wq, dtype=np.float32)
    wk = np.asarray(wk, dtype=np.float32)
    wv = np.asarray(wv, dtype=np.float32)

    if "nc" not in _PROGRAM_CACHE:
        _PROGRAM_CACHE["nc"] = _build_program()
    nc = _PROGRAM_CACHE["nc"]

    c2, s2, tri = _host_tables()
    scale = np.float32(1.0 / np.sqrt(DK))

    xp = [_pack_x(x[b]) for b in range(B)]
    in_maps = []
    for c in range(N_CORES):
        b, hg = divmod(c, HPC)
        rows = slice(hg * FPC, (hg + 1) * FPC)
        im = {
            "wq": _pack_w_headmajor(wq[rows] * scale),
            "wk": _pack_w_headmajor(wk[rows]),
            "wv": _pack_w_ktmajor(wv[rows]),
            "c2": c2, "s2": s2, "tri": tri,
        }
        for i in range(NTB):
            im[f"x{i}"] = xp[b][i]
        in_maps.append(im)

    last_err = None
    for attempt in range(3):
        try:
            res = run_bass_kernel_spmd(nc, in_maps, list(range(N_CORES)),
                                       **_PROGRAM_CACHE.get("run_kwargs", {}))
            break
        except Exception as e:  # transient NRT device errors recover on retry
            last_err = e
            time.sleep(2.0)
    else:
        raise last_err
    _PROGRAM_CACHE["last_results"] = res

    out = np.empty((B, S, D), np.float32)
    for c in range(N_CORES):
        b, hg = divmod(c, HPC)
        out[b, :, hg * FPC:(hg + 1) * FPC] = res.results[c]["out"]
    return out
